# revision 1
# baseline (speedup 1.0000x reference)
"""Trainium2 Bass kernel for: ConvTranspose2d(128->256, k=4, s=2, p=1)
-> MaxPool2d(2,2) -> Hardtanh -> spatial mean -> Tanh.

Key algebraic restructuring: the stride-2 transposed conv decomposes into 4
polyphase 2x2 convolutions, and the outputs of the 4 phases at pooled
position (i, j) are exactly the 4 elements of the 2x2 maxpool window at
(i, j).  So the 128x128 conv-transpose output is never materialized:

    pooled[b, co, i, j] = max_phase  conv2x2_phase(x)[b, co, i, j] + bias

Everything stays at 64x64 resolution.  Each phase conv is 4 accumulating
K=128 matmuls on the PE array (Cin on partitions, Cout in two 128-halves).
The taps' spatial shifts are realized as AP offsets into a zero-padded
66x66 "canvas" copy of the image in SBUF; the moving operand is a
[8 rows x 64 cols] strided view so only valid output columns are computed
and each 8-row chunk exactly fills one PSUM bank (512 fp32).

Sharding: data-parallel over batch, 8 images per core on 8 cores.
Weights (tiny) replicated.  Matmuls in bf16 (fp32 matmul is 2x slower on
the PE; bf16 keeps ~2e-4 relative error here), accumulation in fp32 PSUM,
phase-max/clip tree in bf16 on DVE, mean+tanh in fp32 on ACT/DVE.
"""

from contextlib import ExitStack

import ml_dtypes
import numpy as np

import concourse.bacc as bacc
import concourse.bass as bass
import concourse.mybir as mybir
import concourse.tile as tile
from concourse.bass_utils import run_bass_kernel_spmd

# Problem dims (hardcoded per contract)
B, CIN, COUT, H, W = 64, 128, 256, 64, 64
NCORES = 8
BPC = B // NCORES  # images per core

WP = 66  # padded row width (1 + 64 + 1)
NROW = 66  # padded rows (1 + 64 + 1)
CVTOT = WP * NROW  # 4356

# Output rows r=1..64 of the canvas grid, 8 chunks x 8 rows; each chunk's
# [8 x 64] valid-column block exactly fills one PSUM bank.
NCHUNK = 8
CHUNK_ROWS = [8] * NCHUNK
CHUNK_R0 = [1 + 8 * i for i in range(NCHUNK)]
GROUPS = [[0, 1, 2, 3], [4, 5, 6, 7]]

F32 = mybir.dt.float32
BF16 = mybir.dt.bfloat16


def _tap(ph: int, a: int):
    """For phase parity ph (0=even output coord, 1=odd) and tap index a,
    return (input shift, kernel index) in one dimension.

    ConvTranspose2d(stride=2, pad=1): out[2q+r] = sum over taps of
    x[q+di] * w[k].  r=0: (di,k) in {(0,1), (-1,3)}; r=1: {(1,0), (0,2)}.
    """
    if ph == 0:
        return (0, 1) if a == 0 else (-1, 3)
    return (1, 0) if a == 0 else (0, 2)


def _wcol(half: int, p: int, t: int) -> int:
    return ((half * 4 + p) * 4 + t) * 128


def build_nc(
    n_imgs: int = BPC,
    n_halves: int = 2,
    groups=None,
    repeat: int = 1,
    fine_psum: bool = False,
    tap_outer: bool = True,
    deep_bufs: bool = False,
) -> bass.Bass:
    """repeat>1 wraps the whole compute in a hardware loop executing it
    `repeat` times — used only for wall-clock timing (amortizes the ~80ms
    axon RPC overhead); the graded path uses repeat=1 (no loop)."""
    if groups is None:
        groups = GROUPS
    nc = bacc.Bacc("TRN2", target_bir_lowering=False, debug=False)

    xc = nc.dram_tensor("xc", [BPC, 128, CVTOT], BF16, kind="ExternalInput")
    wm = nc.dram_tensor("wm", [128, 2 * 4 * 4 * 128], BF16, kind="ExternalInput")
    br = nc.dram_tensor("br", [128, 2], F32, kind="ExternalInput")
    out = nc.dram_tensor("out", [128, 2 * BPC], F32, kind="ExternalOutput")

    Id = mybir.ActivationFunctionType.Identity
    Tanh = mybir.ActivationFunctionType.Tanh
    MAX = mybir.AluOpType.max
    MIN = mybir.AluOpType.min

    with ExitStack() as ctx:
        tc = ctx.enter_context(tile.TileContext(nc))
        consts = ctx.enter_context(tc.tile_pool(name="consts", bufs=1))
        canvp = ctx.enter_context(
            tc.tile_pool(name="canv", bufs=4 if deep_bufs else 3)
        )
        psump = ctx.enter_context(
            tc.tile_pool(name="ps", bufs=4 if fine_psum else 2, space="PSUM")
        )
        evacp = ctx.enter_context(
            tc.tile_pool(name="ev", bufs=12 if deep_bufs else 8)
        )
        mpool = ctx.enter_context(
            tc.tile_pool(name="mt", bufs=4 if deep_bufs else 3)
        )
        accp = ctx.enter_context(
            tc.tile_pool(name="acc", bufs=6 if deep_bufs else 4)
        )

        w_sb = consts.tile([128, 2 * 4 * 4 * 128], BF16, tag="w")
        nc.sync.dma_start(w_sb[:], wm[:, :])
        b_sb = consts.tile([128, 2], F32, tag="b")
        nc.sync.dma_start(b_sb[:], br[:, :])
        s_all = consts.tile([128, 2 * BPC], F32, tag="sums")
        nc.vector.memset(s_all[:], 0.0)
        o_sb = consts.tile([128, 2 * BPC], F32, tag="out")

        def body():
            for img in range(n_imgs):
                canv = canvp.tile([128, CVTOT], BF16, tag="canv")
                nc.sync.dma_start(canv[:], xc[img])
                cv = canv[:].rearrange("p (r c) -> p r c", c=WP)
                for half in range(n_halves):
                    acc = accp.tile([128, len(groups)], F32, tag="acc")
                    for g, chunks in enumerate(groups):
                        nch = len(chunks)
                        evs = []
                        for p in range(4):
                            ph, pw = p >> 1, p & 1
                            if fine_psum:
                                # two 2-bank tiles per phase: finer-grained
                                # WAR release so PE's start-of-phase matmuls
                                # rarely wait on a whole 16-MM evac.
                                psa = psump.tile([128, 2, 512], F32, tag="ps")
                                psb = psump.tile([128, 2, 512], F32, tag="ps")
                                pss = [psa, psb]
                            else:
                                ps4 = psump.tile([128, 4, 512], F32, tag="ps")
                            if tap_outer:
                                mm_iter = [
                                    (ci, ch, t)
                                    for t in range(4)
                                    for ci, ch in enumerate(chunks)
                                ]
                            else:
                                mm_iter = [
                                    (ci, ch, t)
                                    for ci, ch in enumerate(chunks)
                                    for t in range(4)
                                ]
                            for ci, ch, t in mm_iter:
                                r0 = CHUNK_R0[ch]
                                nr = CHUNK_ROWS[ch]
                                dst = (
                                    pss[ci // 2][:, ci % 2, : nr * 64]
                                    if fine_psum
                                    else ps4[:, ci, : nr * 64]
                                )
                                a, bb = t >> 1, t & 1
                                di, _kh = _tap(ph, a)
                                dj, _kw = _tap(pw, bb)
                                col = _wcol(half, p, t)
                                nc.tensor.matmul(
                                    dst,
                                    w_sb[:, col : col + 128],
                                    cv[:, r0 + di : r0 + di + nr, 1 + dj : 65 + dj],
                                    start=(t == 0),
                                    stop=(t == 3),
                                    skip_group_check=tap_outer,
                                )
                            # Evacuate this phase's group (PSUM fp32 -> SBUF
                            # bf16) on ScalarE with the bias add fused in.
                            ev = evacp.tile([128, 4, 512], BF16, tag="ev")
                            if fine_psum:
                                for j in range(2):
                                    nc.scalar.activation(
                                        ev[:, 2 * j : 2 * j + 2, :],
                                        pss[j][:, :, :],
                                        Id,
                                        bias=b_sb[:, half : half + 1],
                                    )
                            else:
                                nc.scalar.activation(
                                    ev[:, :nch, :],
                                    ps4[:, :nch, :],
                                    Id,
                                    bias=b_sb[:, half : half + 1],
                                )
                            evs.append(ev)

                        # max over the 4 phases (= the 2x2 maxpool), then
                        # hardtanh clip, then sum -> one fp32 partial per group.
                        nf = nch * 512
                        m01 = mpool.tile([128, 4 * 512], BF16, tag="m01")
                        nc.vector.tensor_tensor(
                            m01[:, :nf],
                            evs[0][:].rearrange("p a b -> p (a b)")[:, :nf],
                            evs[1][:].rearrange("p a b -> p (a b)")[:, :nf],
                            MAX,
                        )
                        m23 = mpool.tile([128, 4 * 512], BF16, tag="m23")
                        nc.vector.tensor_tensor(
                            m23[:, :nf],
                            evs[2][:].rearrange("p a b -> p (a b)")[:, :nf],
                            evs[3][:].rearrange("p a b -> p (a b)")[:, :nf],
                            MAX,
                        )
                        nc.vector.tensor_tensor(m01[:, :nf], m01[:, :nf], m23[:, :nf], MAX)
                        nc.vector.tensor_scalar(
                            out=m01[:, :nf],
                            in0=m01[:, :nf],
                            scalar1=-1.0,
                            scalar2=1.0,
                            op0=MAX,
                            op1=MIN,
                        )
                        nc.vector.tensor_reduce(
                            acc[:, g : g + 1],
                            m01[:, :nf],
                            axis=mybir.AxisListType.X,
                            op=mybir.AluOpType.add,
                        )
                    idx = img * 2 + half
                    nc.vector.reduce_sum(
                        s_all[:, idx : idx + 1],
                        acc[:, : len(groups)],
                        axis=mybir.AxisListType.X,
                    )


        if repeat > 1:
            with tc.For_i(0, repeat, 1):
                body()
        else:
            body()

        nc.scalar.activation(o_sb[:], s_all[:], Tanh, scale=1.0 / 4096.0)
        nc.sync.dma_start(out[:, :], o_sb[:])

    nc.finalize()
    return nc


_CACHE: dict = {}


def _get_nc() -> bass.Bass:
    if "nc" not in _CACHE:
        _CACHE["nc"] = build_nc()
    return _CACHE["nc"]


def make_in_maps(x: np.ndarray, weight: np.ndarray, bias: np.ndarray):
    x = np.asarray(x, dtype=np.float32)
    weight = np.asarray(weight, dtype=np.float32)
    bias = np.asarray(bias, dtype=np.float32)

    canv = np.zeros((B, 128, CVTOT), dtype=ml_dtypes.bfloat16)
    view = canv.reshape(B, 128, NROW, WP)
    view[:, :, 1:65, 1:65] = x  # cast fp32 -> bf16

    wmv = np.zeros((128, 2 * 4 * 4 * 128), dtype=ml_dtypes.bfloat16)
    for half in range(2):
        for p in range(4):
            ph, pw = p >> 1, p & 1
            for t in range(4):
                a, bb = t >> 1, t & 1
                _di, kh = _tap(ph, a)
                _dj, kw = _tap(pw, bb)
                col = _wcol(half, p, t)
                wmv[:, col : col + 128] = weight[
                    :, half * 128 : (half + 1) * 128, kh, kw
                ]

    brv = np.ascontiguousarray(bias.reshape(2, 128).T, dtype=np.float32)

    return [
        {"xc": canv[c * BPC : (c + 1) * BPC], "wm": wmv, "br": brv}
        for c in range(NCORES)
    ]


def assemble_output(results: list) -> np.ndarray:
    outs = []
    for c in range(NCORES):
        o = np.asarray(results[c]["out"])  # [128, 2*BPC]
        o = o.reshape(128, BPC, 2).transpose(1, 2, 0).reshape(BPC, COUT)
        outs.append(o)
    return np.concatenate(outs, 0).reshape(B, COUT, 1, 1).astype(np.float32)


def kernel(x: np.ndarray, weight: np.ndarray, bias: np.ndarray) -> np.ndarray:
    nc = _get_nc()
    in_maps = make_in_maps(x, weight, bias)
    res = run_bass_kernel_spmd(nc, in_maps, core_ids=list(range(NCORES)))
    return assemble_output(res.results)



# revision 5
# speedup vs baseline: 1.8809x; 1.8809x over previous
"""Trainium2 Bass kernel for: ConvTranspose2d(128->256, k=4, s=2, p=1)
-> MaxPool2d(2,2) -> Hardtanh -> spatial mean -> Tanh.

Polyphase decomposition (as the bf16 baseline): the stride-2 transposed conv
splits into 4 polyphase 2x2 convolutions whose outputs at pooled position
(i, j) are exactly the 4 elements of the 2x2 maxpool window, so everything
stays at 64x64 and the 128x128 conv output is never materialized.

This version gets ~2x more PE throughput from fp8(e4m3) matmuls in DoubleRow
perf mode: the PE virtualizes to 128x256 (two fp8 weights per cell), so the
two ROW taps of each phase become a single matmul with contraction 256.  The
moving operand is a 3D AP [cin=128, pair=2, free] over a zero-padded 66x66
canvas where the pair dim strides one canvas row (+66) and the column tap is
a flat +-1 offset; the free dim covers whole 66-wide rows (7 rows = 462 <=
512 fp32 = one PSUM bank) including 2 ignorable pad columns per row.

Weights are pre-scaled by S=64 before the fp8 cast (w std ~0.022 would land
in e4m3's subnormal range); the scale and the conv bias are folded into
per-channel Hardtanh clip bounds lo = S*(-1-b), hi = S*(1-b) and into the
final Tanh's scale/bias, so no separate bias-add pass exists.

Downstream per chunk (4 phase banks in PSUM):
  max01 = TT-max of bank pairs      (DVE; from PSUM directly, or from a
                                     bf16 ACT evacuation - split tunable)
  s2    = (max01a max lo) max max01b   one fused scalar_tensor_tensor
  sum  += min(s2, hi)                  one tensor_scalar with add-accum
Final: out = Tanh(sum/(4096*S) + b) on ACT.

Sharding: data-parallel over batch, 8 images per core on 8 cores.
"""

from contextlib import ExitStack

import ml_dtypes
import numpy as np

import concourse.bacc as bacc
import concourse.bass as bass
import concourse.mybir as mybir
import concourse.tile as tile
from concourse.bass_utils import run_bass_kernel_spmd

# Problem dims (hardcoded per contract)
B, CIN, COUT, H, W = 64, 128, 256, 64, 64
NCORES = 8
BPC = B // NCORES  # images per core

WP = 66  # padded row width (1 + 64 + 1)
NROW = 66  # padded rows (1 + 64 + 1)
CVT = WP * NROW + 2  # lead/tail guard bytes for the dj=+-1 flat offsets

# Output rows 1..64 of the canvas grid: 9 chunks of 7 rows + 1 tail row.
CHUNKS = [(1 + 7 * i, 7) for i in range(9)] + [(64, 1)]
NCH = len(CHUNKS)

WSCALE = 64.0  # weight pre-scale before fp8 cast

F32 = mybir.dt.float32
BF16 = mybir.dt.bfloat16
FP8 = mybir.dt.float8e4

# Per (img, half): chunk indices whose phase-max reads banks 2:4 from PSUM
# on DVE (1x mode) with only a half ACT evacuation - balances ACT vs DVE.
DIRECT_CHUNKS = (1, 4, 7)


def _tap(ph: int, a: int):
    """For phase parity ph (0=even output coord, 1=odd) and tap index a,
    return (input shift, kernel index) in one dimension.

    ConvTranspose2d(stride=2, pad=1): out[2q+r] = sum over taps of
    x[q+di] * w[k].  r=0: (di,k) in {(0,1), (-1,3)}; r=1: {(1,0), (0,2)}.
    """
    if ph == 0:
        return (0, 1) if a == 0 else (-1, 3)
    return (1, 0) if a == 0 else (0, 2)


def build_nc(
    n_imgs: int = BPC,
    repeat: int = 1,
    direct_chunks=DIRECT_CHUNKS,
    perf_mode=None,
) -> bass.Bass:
    """repeat>1 wraps the whole compute in a hardware loop executing it
    `repeat` times - used only for wall-clock timing (amortizes the ~80ms
    axon RPC overhead); the graded path uses repeat=1 (no loop)."""
    if perf_mode is None:
        perf_mode = mybir.MatmulPerfMode.DoubleRow
    nc = bacc.Bacc("TRN2", target_bir_lowering=False, debug=False)

    xc = nc.dram_tensor("xc", [BPC, 128, CVT], FP8, kind="ExternalInput")
    wm = nc.dram_tensor("wm", [128, 16 * 2 * 128], FP8, kind="ExternalInput")
    br = nc.dram_tensor("br", [128, 2], F32, kind="ExternalInput")
    clo = nc.dram_tensor("clo", [128, 2], F32, kind="ExternalInput")
    chi = nc.dram_tensor("chi", [128, 2], F32, kind="ExternalInput")
    out = nc.dram_tensor("out", [128, 2 * BPC], F32, kind="ExternalOutput")

    Copy = mybir.ActivationFunctionType.Copy
    Tanh = mybir.ActivationFunctionType.Tanh
    MAX = mybir.AluOpType.max
    MIN = mybir.AluOpType.min
    ADD = mybir.AluOpType.add

    with ExitStack() as ctx:
        tc = ctx.enter_context(tile.TileContext(nc))
        consts = ctx.enter_context(tc.tile_pool(name="consts", bufs=1))
        canvp = ctx.enter_context(tc.tile_pool(name="canv", bufs=3))
        psump = ctx.enter_context(tc.tile_pool(name="ps", bufs=2, space="PSUM"))
        evacp = ctx.enter_context(tc.tile_pool(name="ev", bufs=3))
        mpool = ctx.enter_context(tc.tile_pool(name="mt", bufs=3))
        s2pool = ctx.enter_context(tc.tile_pool(name="s2", bufs=3))
        cpool = ctx.enter_context(tc.tile_pool(name="ct", bufs=3))
        accp = ctx.enter_context(tc.tile_pool(name="acc", bufs=3))

        w_sb = consts.tile([128, 16, 2, 128], FP8, tag="w")
        nc.sync.dma_start(
            w_sb[:].rearrange("p a b c -> p (a b c)"), wm[:, :]
        )
        b_sb = consts.tile([128, 2], F32, tag="b")
        nc.sync.dma_start(b_sb[:], br[:, :])
        lo_sb = consts.tile([128, 2], F32, tag="lo")
        nc.sync.dma_start(lo_sb[:], clo[:, :])
        hi_sb = consts.tile([128, 2], F32, tag="hi")
        nc.sync.dma_start(hi_sb[:], chi[:, :])
        s_all = consts.tile([128, 2 * BPC], F32, tag="sums")
        nc.vector.memset(s_all[:], 0.0)
        o_sb = consts.tile([128, 2 * BPC], F32, tag="out")
        nc.vector.memset(o_sb[:], 0.0)

        def body():
            for img in range(n_imgs):
                canv = canvp.tile([128, CVT], FP8, tag="canv")
                nc.sync.dma_start(canv[:], xc[img])
                for half in range(2):
                    acc = accp.tile([128, NCH], F32, tag="acc")
                    for ci, (r0, nr) in enumerate(CHUNKS):
                        nf = nr * WP
                        nv = nr * 64
                        ps = psump.tile([128, 4, 512], F32, tag="ps")
                        for p in range(4):
                            ph, pw = p >> 1, p & 1
                            di0 = _tap(ph, 0)[0]
                            rowbase = r0 + di0 - 1
                            for bb in range(2):
                                dj = _tap(pw, bb)[0]
                                base = 1 + rowbase * WP + dj
                                v = canv[:, base : base + nf].unsqueeze(1)
                                v.ap[1] = [WP, 2]
                                nc.tensor.matmul(
                                    ps[:, p, :nf],
                                    w_sb[:, (half * 4 + p) * 2 + bb],
                                    v,
                                    start=(bb == 0),
                                    stop=(bb == 1),
                                    perf_mode=perf_mode,
                                )
                        # phase-max pair stage.  HW allows only one PSUM
                        # input per DVE op, so the "direct" variant ACT-evacs
                        # banks 0:2 (ready early, overlaps phase-2/3 matmuls)
                        # and maxes them against banks 2:4 read from PSUM.
                        m = mpool.tile([128, 2, 7, 64], BF16, tag="m")
                        if ci in direct_chunks and nr == 7:
                            ev = evacp.tile([128, 4, 7, 64], BF16, tag="ev")
                            pva = ps[:, 0:2, 1:65].unsqueeze(2)
                            pva.ap[2] = [WP, nr]
                            nc.scalar.activation(ev[:, 0:2, :nr, :], pva, Copy)
                            pvb = ps[:, 2:4, 1:65].unsqueeze(2)
                            pvb.ap[2] = [WP, nr]
                            nc.vector.tensor_tensor(
                                m[:, :, :nr, :], pvb, ev[:, 0:2, :nr, :], MAX
                            )
                        else:
                            ev = evacp.tile([128, 4, 7, 64], BF16, tag="ev")
                            pv = ps[:, 0:4, 1:65].unsqueeze(2)
                            pv.ap[2] = [WP, nr]
                            nc.scalar.activation(ev[:, :, :nr, :], pv, Copy)
                            nc.vector.tensor_tensor(
                                m[:, :, :nr, :],
                                ev[:, 0:2, :nr, :],
                                ev[:, 2:4, :nr, :],
                                MAX,
                            )
                        # (max . lower-clip) then (upper-clip . sum-accum)
                        s2 = s2pool.tile([128, 7, 64], BF16, tag="s2")
                        nc.vector.scalar_tensor_tensor(
                            s2[:, :nr, :],
                            m[:, 0, :nr, :],
                            lo_sb[:, half : half + 1],
                            m[:, 1, :nr, :],
                            MAX,
                            MAX,
                        )
                        cs = cpool.tile([128, 7, 64], BF16, tag="c")
                        nc.vector.tensor_scalar(
                            out=cs[:, :nr, :],
                            in0=s2[:, :nr, :],
                            scalar1=hi_sb[:, half : half + 1],
                            scalar2=None,
                            op0=MIN,
                            op1=ADD,
                            accum_out=acc[:, ci : ci + 1],
                        )
                    idx = half * n_imgs + img
                    nc.vector.reduce_sum(
                        s_all[:, idx : idx + 1],
                        acc[:, :NCH],
                        axis=mybir.AxisListType.X,
                    )

        if repeat > 1:
            with tc.For_i(0, repeat, 1):
                body()
        else:
            body()

        for half in range(2):
            sl = slice(half * n_imgs, (half + 1) * n_imgs)
            nc.scalar.activation(
                o_sb[:, sl],
                s_all[:, sl],
                Tanh,
                bias=b_sb[:, half : half + 1],
                scale=1.0 / (4096.0 * WSCALE),
            )
        nc.sync.dma_start(out[:, :], o_sb[:])

    nc.finalize()
    return nc


_CACHE: dict = {}


def _get_nc() -> bass.Bass:
    if "nc" not in _CACHE:
        _CACHE["nc"] = build_nc()
    return _CACHE["nc"]


def make_in_maps(x: np.ndarray, weight: np.ndarray, bias: np.ndarray):
    x = np.asarray(x, dtype=np.float32)
    weight = np.asarray(weight, dtype=np.float32)
    bias = np.asarray(bias, dtype=np.float32)

    canv = np.zeros((B, 128, CVT), dtype=ml_dtypes.float8_e4m3)
    view = canv[:, :, 1 : 1 + WP * NROW].reshape(B, 128, NROW, WP)
    view[:, :, 1:65, 1:65] = x  # cast fp32 -> fp8

    # weight pairs: pair i=0 is the a=1 row tap (one canvas row up),
    # i=1 the a=0 row tap, matching the moving AP's +WP pair stride.
    wmv = np.zeros((128, 16 * 2 * 128), dtype=ml_dtypes.float8_e4m3)
    for half in range(2):
        for p in range(4):
            ph, pw = p >> 1, p & 1
            for bb in range(2):
                kw = _tap(pw, bb)[1]
                for i in range(2):
                    kh = _tap(ph, 1 - i)[1]
                    col = ((((half * 4 + p) * 2 + bb) * 2) + i) * 128
                    wmv[:, col : col + 128] = (
                        WSCALE * weight[:, half * 128 : (half + 1) * 128, kh, kw]
                    )

    brv = np.ascontiguousarray(bias.reshape(2, 128).T, dtype=np.float32)
    clov = np.ascontiguousarray(WSCALE * (-1.0 - brv), dtype=np.float32)
    chiv = np.ascontiguousarray(WSCALE * (1.0 - brv), dtype=np.float32)

    return [
        {
            "xc": canv[c * BPC : (c + 1) * BPC],
            "wm": wmv,
            "br": brv,
            "clo": clov,
            "chi": chiv,
        }
        for c in range(NCORES)
    ]


def assemble_output(results: list) -> np.ndarray:
    outs = []
    for c in range(NCORES):
        o = np.asarray(results[c]["out"])  # [128, 2*BPC] = [part, half, img]
        o = o.reshape(128, 2, BPC).transpose(2, 1, 0).reshape(BPC, COUT)
        outs.append(o)
    return np.concatenate(outs, 0).reshape(B, COUT, 1, 1).astype(np.float32)


def kernel(x: np.ndarray, weight: np.ndarray, bias: np.ndarray) -> np.ndarray:
    nc = _get_nc()
    in_maps = make_in_maps(x, weight, bias)
    res = run_bass_kernel_spmd(nc, in_maps, core_ids=list(range(NCORES)))
    return assemble_output(res.results)
